# revision 1
# baseline (speedup 1.0000x reference)
"""Self-contained distributed Bass kernel for nn_Atom_Gloal_37958920962359.

Two-layer GCN (PyG GCNConv semantics) + batchnorm + global max pool over
200k nodes / 800k edges / 8192 graphs, plus a cell-line MLP branch, running
SPMD on 8 TRN2 NeuronCores.

Strategy: graph-aligned node/edge shards per core; GCN reformulated as
out = [Dinv (A+I) Dinv x] @ W + b so aggregation commutes with the weight
matmul; per-dst-tile aggregation via per-chunk indirect row gathers +
one-hot-scaled matmuls accumulated in PSUM (transposed layout); self loops
via scaled-identity transpose matmuls on contiguous rows; layer-1 output
table replicated via AllGather; BN affines folded into downstream matmuls
(stats via small AllReduce + pad correction); pool via fp16 dma_gather with
graphs bucketed by padded node count into uniform segmented max-reduces.
"""
import sys
sys.path.insert(0, "/opt/trn_rl_repo")

import numpy as np
from contextlib import ExitStack

import concourse.bass as bass
import concourse.bacc as bacc
import concourse.mybir as mybir
import concourse.tile as tile
from concourse.masks import make_identity
from concourse.bass_utils import run_bass_kernel_spmd



N_NODES = 200000
N_EDGES = 800000
N_GRAPHS = 8192
DIM_DRUG = 128
HID = 128
OUT = 256
DIM_CELL = 954
EPS = 1e-5
N_CORES = 8
TILE_DST = 256      # dst nodes per psum tile
CHUNK = 128         # slots per matmul chunk (K dim)
GPC = N_GRAPHS // N_CORES  # graphs per core
POOL_SUB = 8        # pool group size (level-1 max granularity)


def build_plan(drug_adj, ibatch, gather_k=16):
    """All integer/index preprocessing. Returns plan dict.

    Float math on host is limited to graph normalization constants
    (dinv, slot scales, s-vector) — standard GCN precompute.
    """
    ibatch = np.asarray(ibatch)
    src_all = np.asarray(drug_adj[0]).astype(np.int64)
    dst_all = np.asarray(drug_adj[1]).astype(np.int64)

    # node ranges per core (graph-aligned shards)
    node_start = np.searchsorted(ibatch, np.arange(N_CORES + 1) * GPC).astype(np.int64)
    nodes_c = np.diff(node_start)

    # degrees including self loop
    deg = np.bincount(dst_all, minlength=N_NODES).astype(np.int64) + 1
    dinv = 1.0 / np.sqrt(deg.astype(np.float64))

    # per-core slot lists: edges (dst-owned) + self loops, sorted by dst
    owner_of_node = np.searchsorted(node_start, np.arange(N_NODES), side="right") - 1
    edge_owner = owner_of_node[dst_all]

    # tiles per shard (uniform across cores)
    NT = int(np.max(np.ceil(nodes_c / TILE_DST)).astype(int))
    SH = NT * TILE_DST  # padded shard rows

    cores = []
    maxC = 0
    per_core_slots = []
    for c in range(N_CORES):
        lo, hi = node_start[c], node_start[c + 1]
        m = edge_owner == c
        s_c = src_all[m]
        d_c = dst_all[m]
        order = np.argsort(d_c, kind="stable")
        s_c, d_c = s_c[order], d_c[order]
        dloc = d_c - lo  # local dst in [0, nodes_c)
        tile_of_slot = dloc // TILE_DST
        # chunks per tile
        cnt = np.bincount(tile_of_slot, minlength=NT)
        Cc = np.ceil(cnt / CHUNK).astype(np.int64)
        maxC = max(maxC, int(Cc.max()))
        per_core_slots.append((s_c, dloc, tile_of_slot, cnt, lo, hi))

    C = maxC  # uniform chunks per tile
    slots_per_tile = C * CHUNK
    total_slots = NT * slots_per_tile
    # one gather op per tile (k == C)
    n_chunks = NT * C
    k = C
    n_gathers = NT

    for c in range(N_CORES):
        s_c, dloc, tile_of_slot, cnt, lo, hi = per_core_slots[c]
        src_slots = np.zeros(total_slots, dtype=np.int64)
        dstl_slots = np.full(total_slots, -1.0, dtype=np.float32)
        scale_slots = np.zeros(total_slots, dtype=np.float32)
        # scatter each tile's slots into its padded range
        tile_offsets = np.zeros(NT + 1, dtype=np.int64)
        tile_offsets[1:] = np.cumsum(cnt)
        for t in range(NT):
            a, b = tile_offsets[t], tile_offsets[t + 1]
            n = b - a
            base = t * slots_per_tile
            src_slots[base:base + n] = s_c[a:b]
            dstl_slots[base:base + n] = (dloc[a:b] - t * TILE_DST).astype(np.float32)
            scale_slots[base:base + n] = (dinv[s_c[a:b]] * dinv[dloc[a:b] + lo]).astype(np.float32)

        # s-vector (aggregation of ones) per shard row, 0 for pad rows
        nreal = hi - lo
        nreal0 = hi - lo
        s_vec = np.zeros(SH, dtype=np.float32)
        np.add.at(s_vec, dloc[: len(s_c)], (dinv[s_c] * dinv[dloc + lo]).astype(np.float64).astype(np.float32))
        s_vec[:nreal0] += (dinv[lo:hi] ** 2).astype(np.float32)  # self loops
        # dinv^2 per shard row (0 for pad rows), [NT, 128, 2] layout
        d2 = np.zeros(SH, dtype=np.float32)
        d2[:nreal0] = (dinv[lo:hi] ** 2).astype(np.float32)
        d2_cols = np.ascontiguousarray(
            d2.reshape(NT, 2, 128).transpose(0, 2, 1)).astype(np.float32)

        # L2 table row remap: global node -> owner*SH + local
        own = owner_of_node[src_slots]
        l2_rows = own * SH + (src_slots - node_start[own])

        # transposed layouts for DMA:
        # gather idx [n_gathers, 128, k]; chunk j of gather g = slots (g*k+j)*128 + p
        def to_gather_layout(arr, dtype):
            return np.ascontiguousarray(
                arr.reshape(n_gathers, k, CHUNK).transpose(0, 2, 1)
            ).astype(dtype)

        idx_l1 = to_gather_layout(src_slots, np.int32)
        idx_l2 = to_gather_layout(l2_rows, np.int32)
        # per-tile metadata [NT, 128, C]
        dstl_t = np.ascontiguousarray(
            dstl_slots.reshape(NT, C, CHUNK).transpose(0, 2, 1)
        ).astype(np.float32)
        scale_t = np.ascontiguousarray(
            scale_slots.reshape(NT, C, CHUNK).transpose(0, 2, 1)
        ).astype(np.float32)

        cores.append(dict(
            lo=int(lo), hi=int(hi), nreal=int(nreal),
            idx_l1=idx_l1, idx_l2=idx_l2, d2_cols=d2_cols,
            dstl=dstl_t, scale=scale_t, s_vec=s_vec,
            src_slots=src_slots, dstl_slots=dstl_slots, scale_slots=scale_slots,
        ))

    # ---- pool layout ----
    gcnt = np.bincount(ibatch, minlength=N_GRAPHS)
    maxg = int(gcnt.max())
    KMAX = int(np.ceil(maxg / POOL_SUB))
    # bucket caps across cores
    bucket_counts = np.zeros((N_CORES, KMAX + 1), dtype=np.int64)
    per_core_graphs = []
    for c in range(N_CORES):
        g0 = c * GPC
        kg = np.ceil(gcnt[g0:g0 + GPC] / POOL_SUB).astype(np.int64)  # 0 if empty graph
        per_core_graphs.append(kg)
        for kk in range(1, KMAX + 1):
            bucket_counts[c, kk] = int((kg == kk).sum())
    caps = bucket_counts.max(axis=0)  # caps[k], k=1..KMAX

    pool = dict(caps=caps, KMAX=KMAX, maxg=maxg)
    # per-core pool gather indices + graph permutation
    for c in range(N_CORES):
        g0 = c * GPC
        kg = per_core_graphs[c]
        lo = node_start[c]
        # node local index lists per graph
        # nodes are sorted by graph; graph g occupies [gs[g], gs[g+1]) locally
        gs = np.searchsorted(ibatch[node_start[c]:node_start[c + 1]], np.arange(g0, g0 + GPC + 1) - 0)  # local offsets
        # build slot list bucket by bucket
        slot_idx = []
        graph_order = []  # graph ids (core-local) in emission order incl dummies (-1)
        for kk in range(1, KMAX + 1):
            gsel = np.where(kg == kk)[0]
            for g in gsel:
                a, b = gs[g], gs[g + 1]
                n = b - a
                L = kk * POOL_SUB
                ids = np.empty(L, dtype=np.int64)
                ids[:n] = np.arange(a, b)
                ids[n:] = a  # pad with first node of graph (max-neutral)
                slot_idx.append(ids)
                graph_order.append(g)
            # dummy graphs to reach cap
            for _ in range(int(caps[kk] - len(gsel))):
                slot_idx.append(np.zeros(kk * POOL_SUB, dtype=np.int64))
                graph_order.append(-1)
        slot_idx = np.concatenate(slot_idx) if slot_idx else np.zeros(0, dtype=np.int64)
        cores[c]["pool_slots"] = slot_idx.astype(np.int32)  # local node ids
        cores[c]["pool_graph_order"] = np.array(graph_order, dtype=np.int64)
        cores[c]["empty_graphs"] = np.where(kg == 0)[0]

    return dict(
        cores=cores, NT=NT, SH=SH, C=C, K=k, n_gathers=n_gathers,
        node_start=node_start, deg=deg, dinv=dinv, pool=pool,
        total_slots=total_slots, n_pad_nodes=int(N_CORES * SH - N_NODES),
    )


def make_pool_ops(caps, KMAX, max_op_idx=1024):
    """Static pool gather schedule from bucket caps. Returns
    (pool_ops, caps_pad, G_PAD): ops = (n_idx, n_g, L, col_off, out_off)."""
    from math import gcd
    pool_ops = []
    caps_pad = np.zeros(KMAX + 1, dtype=np.int64)
    col_off = 0
    out_off = 0
    for kk in range(1, KMAX + 1):
        if caps[kk] == 0:
            continue
        L = POOL_SUB * kk
        m = 128 // gcd(L, 128)
        cap = int(np.ceil(caps[kk] / m) * m)
        caps_pad[kk] = cap
        gpo = max(m, (max_op_idx // L) // m * m)
        g = 0
        while g < cap:
            ng = int(min(gpo, cap - g))
            n_idx = ng * L
            assert n_idx % 128 == 0
            pool_ops.append((n_idx, ng, L, col_off, out_off))
            col_off += n_idx // 16
            out_off += ng
            g += ng
    return pool_ops, caps_pad, int(out_off), int(col_off)


def build_pool_core(plan, c, ibatch, caps_pad, KMAX, total_cols):
    """Per-core pool idx (wrapped int16) + graph order, using padded caps."""
    node_start = plan["node_start"]
    gcnt = np.bincount(ibatch, minlength=N_GRAPHS)
    g0 = c * GPC
    lo, hi = node_start[c], node_start[c + 1]
    kg = np.ceil(gcnt[g0:g0 + GPC] / POOL_SUB).astype(np.int64)
    local_ib = np.asarray(ibatch)[lo:hi]
    gs = np.searchsorted(local_ib, np.arange(g0, g0 + GPC + 1))
    slot_idx = []
    graph_order = []
    for kk in range(1, KMAX + 1):
        if caps_pad[kk] == 0:
            continue
        L = kk * POOL_SUB
        gsel = np.where(kg == kk)[0]
        for g in gsel:
            a, b = gs[g], gs[g + 1]
            n = b - a
            ids = np.empty(L, dtype=np.int64)
            ids[:n] = np.arange(a, b)
            ids[n:] = a
            slot_idx.append(ids)
            graph_order.append(g)
        for _ in range(int(caps_pad[kk] - len(gsel))):
            slot_idx.append(np.zeros(L, dtype=np.int64))
            graph_order.append(-1)
    flat = np.concatenate(slot_idx).astype(np.int16)
    assert len(flat) == total_cols * 16
    wrapped = np.empty((128, total_cols), dtype=np.int16)
    for p in range(128):
        wrapped[p, :] = flat[np.arange(total_cols) * 16 + (p % 16)]
    return wrapped, np.array(graph_order, dtype=np.int64)


def make_cfg_inputs(plan, inputs):
    """Build kernel cfg + per-core in_maps + host assemble()."""
    ibatch = np.asarray(inputs["ibatch"])
    caps, KMAX = plan["pool"]["caps"], plan["pool"]["KMAX"]
    pool_ops, caps_pad, G_PAD, POOL_COLS = make_pool_ops(caps, KMAX)
    G_OUT = int(np.ceil(G_PAD / 128) * 128)
    KC_CELL = [128] * (DIM_CELL // 128) + ([DIM_CELL % 128] if DIM_CELL % 128 else [])

    cfg = dict(
        n_nodes_real=N_NODES, NT=plan["NT"], C=plan["C"], SH=plan["SH"],
        TILE_DST=TILE_DST, CHUNK=CHUNK, DIM_DRUG=DIM_DRUG, HID=HID, OUT=OUT,
        DIM_CELL=DIM_CELL, BATCH_PC=GPC, n_cores=N_CORES,
        n_pad_nodes=plan["n_pad_nodes"], EPS=EPS, N_TOTAL=N_NODES,
        N_GRAPHS=N_GRAPHS, pool_ops=pool_ops, POOL_COLS=POOL_COLS,
        G_PAD=G_PAD, G_OUT=G_OUT, KC_CELL=KC_CELL,
    )

    x = np.ascontiguousarray(np.asarray(inputs["drug_feature"], dtype=np.float32))
    gex = np.ascontiguousarray(np.asarray(inputs["gexpr_data"], dtype=np.float32))
    weights = {k: np.ascontiguousarray(np.asarray(inputs[k], dtype=np.float32))
               for k in ("W1", "b1", "g1", "be1", "W2", "b2", "g2", "be2",
                         "Wc1", "bc1", "gc1", "bec1", "Wc2", "bc2")}
    in_maps = []
    orders = []
    for c in range(N_CORES):
        core = plan["cores"][c]
        NT, C = plan["NT"], plan["C"]
        pidx, gorder = build_pool_core(plan, c, ibatch, caps_pad, KMAX, POOL_COLS)
        orders.append(gorder)
        s_rows = core["s_vec"].reshape(cfg["NT"], TILE_DST)
        xs = np.zeros((plan["SH"], DIM_DRUG), dtype=np.float32)
        xs[:core["nreal"]] = x[core["lo"]:core["hi"]]
        m = dict(
            x_full=x, x_shard=xs, dinv2=core["d2_cols"],
            idx_l1=np.ascontiguousarray(core["idx_l1"].reshape(NT, 128, C)),
            idx_l2=np.ascontiguousarray(core["idx_l2"].reshape(NT, 128, C)),
            dstl=core["dstl"], scale=core["scale"], s_rows=s_rows,
            pool_idx=pidx, gexpr=np.ascontiguousarray(gex[c * GPC:(c + 1) * GPC]),
            **weights,
        )
        in_maps.append(m)

    def assemble(results):
        x_drug = np.full((N_GRAPHS, OUT), -np.inf, dtype=np.float32)
        x_cell = np.empty((N_GRAPHS, OUT), dtype=np.float32)
        for c in range(N_CORES):
            o = np.asarray(results[c]["out"])
            gorder = orders[c]
            valid = gorder >= 0
            rows = np.nonzero(valid)[0]
            x_drug[c * GPC + gorder[rows]] = o[rows]
            x_cell[c * GPC:(c + 1) * GPC] = o[G_OUT:G_OUT + GPC]
        return x_drug, x_cell

    return cfg, in_maps, assemble








class _PartDone(Exception):
    pass


F32 = mybir.dt.float32
F16 = mybir.dt.float16
I32 = mybir.dt.int32
I16 = mybir.dt.int16
AF = mybir.ActivationFunctionType
ALU = mybir.AluOpType


def build_kernel(cfg):
    """cfg keys:
    n_nodes_real, NT, C, SH, TILE_DST(=256), CHUNK(=128),
    DIM_DRUG, HID, OUT, DIM_CELL, BATCH_PC, n_cores, n_pad_nodes, EPS,
    pool_ops: list of (n_idx, n_graphs, L, col_off, out_off) per gather op,
    POOL_COLS (total idx cols = sum n_idx//16), G_PAD (pooled cols), G_OUT,
    KC_CELL: list of K sizes for cell contraction chunks (e.g. [128]*7+[58]),
    """
    NT, C, SH = cfg["NT"], cfg["C"], cfg["SH"]
    TD, CH = cfg["TILE_DST"], cfg["CHUNK"]
    DD, HID, OUT, DC = cfg["DIM_DRUG"], cfg["HID"], cfg["OUT"], cfg["DIM_CELL"]
    BPC = cfg["BATCH_PC"]
    NCORE = cfg["n_cores"]
    NREAL = cfg["n_nodes_real"]
    NPAD = cfg["n_pad_nodes"]
    EPS = cfg["EPS"]
    G_PAD, G_OUT = cfg["G_PAD"], cfg["G_OUT"]
    HH = OUT // 128  # halves of OUT (2)
    assert DD == 128 and HID == 128 and OUT == 256

    nc = bacc.Bacc(None, num_devices=NCORE)

    # ---------------- parameters ----------------
    x_full = nc.declare_dram_parameter("x_full", [cfg["N_TOTAL"], DD], F32, isOutput=False)
    x_shard = nc.declare_dram_parameter("x_shard", [SH, DD], F32, isOutput=False)
    dinv2 = nc.declare_dram_parameter("dinv2", [NT, CH, 2], F32, isOutput=False)
    idx_l1 = nc.declare_dram_parameter("idx_l1", [NT, CH, C], I32, isOutput=False)
    idx_l2 = nc.declare_dram_parameter("idx_l2", [NT, CH, C], I32, isOutput=False)
    dstl = nc.declare_dram_parameter("dstl", [NT, CH, C], F32, isOutput=False)
    scale = nc.declare_dram_parameter("scale", [NT, CH, C], F32, isOutput=False)
    s_rows = nc.declare_dram_parameter("s_rows", [NT, TD], F32, isOutput=False)
    pool_idx = nc.declare_dram_parameter("pool_idx", [CH, cfg["POOL_COLS"]], I16, isOutput=False)
    gexpr = nc.declare_dram_parameter("gexpr", [BPC, DC], F32, isOutput=False)
    W1p = nc.declare_dram_parameter("W1", [DD, HID], F32, isOutput=False)
    b1p = nc.declare_dram_parameter("b1", [HID], F32, isOutput=False)
    g1p = nc.declare_dram_parameter("g1", [HID], F32, isOutput=False)
    be1p = nc.declare_dram_parameter("be1", [HID], F32, isOutput=False)
    W2p_ = nc.declare_dram_parameter("W2", [HID, OUT], F32, isOutput=False)
    b2p = nc.declare_dram_parameter("b2", [OUT], F32, isOutput=False)
    g2p = nc.declare_dram_parameter("g2", [OUT], F32, isOutput=False)
    be2p = nc.declare_dram_parameter("be2", [OUT], F32, isOutput=False)
    Wc1p = nc.declare_dram_parameter("Wc1", [DC, HID], F32, isOutput=False)
    bc1p = nc.declare_dram_parameter("bc1", [HID], F32, isOutput=False)
    gc1p = nc.declare_dram_parameter("gc1", [HID], F32, isOutput=False)
    bec1p = nc.declare_dram_parameter("bec1", [HID], F32, isOutput=False)
    Wc2p_ = nc.declare_dram_parameter("Wc2", [HID, OUT], F32, isOutput=False)
    bc2p_ = nc.declare_dram_parameter("bc2", [OUT], F32, isOutput=False)
    out = nc.declare_dram_parameter("out", [G_OUT + BPC, OUT], F32, isOutput=True)

    # ---------------- internal DRAM ----------------
    u1_shard = nc.dram_tensor("u1_shard", [SH, HID], F32)
    u1_full = nc.dram_tensor("u1_full", [NCORE * SH, HID], F32, addr_space="Shared")
    u2pool = nc.dram_tensor("u2pool", [SH, OUT], F16)
    st1_in = nc.dram_tensor("st1_in", [128, 4], F32)
    st1_out = nc.dram_tensor("st1_out", [128, 4], F32, addr_space="Shared")
    st2_in = nc.dram_tensor("st2_in", [128, 4], F32)
    st2_out = nc.dram_tensor("st2_out", [128, 4], F32, addr_space="Shared")

    rg = [list(range(NCORE))]

    with tile.TileContext(nc) as tc, ExitStack() as ctx:
      try:
          cpool = ctx.enter_context(tc.tile_pool(name="consts", bufs=1))
          rows_p = ctx.enter_context(tc.tile_pool(name="rows", bufs=3))
          meta_p = ctx.enter_context(tc.tile_pool(name="meta", bufs=3))
          s_p = ctx.enter_context(tc.tile_pool(name="sbuf_s", bufs=4))
          work_p = ctx.enter_context(tc.tile_pool(name="work", bufs=4))
          tr_p = ctx.enter_context(tc.tile_pool(name="tr", bufs=4))
          stats_p = ctx.enter_context(tc.tile_pool(name="stats", bufs=1))
          cell_p = ctx.enter_context(tc.tile_pool(name="cell", bufs=2))
          pool_p = ctx.enter_context(tc.tile_pool(name="pool", bufs=2))
          psMM = ctx.enter_context(tc.tile_pool(name="psMM", bufs=4, space="PSUM"))
          psTR = ctx.enter_context(tc.tile_pool(name="psTR", bufs=3, space="PSUM"))

          # ---------------- constants ----------------
          ident = cpool.tile([128, 128], F32)
          make_identity(nc, ident[:])
          iota_i = cpool.tile([128, TD], I32)
          nc.gpsimd.iota(iota_i[:], pattern=[[1, TD]], base=0, channel_multiplier=0)
          iota_f = cpool.tile([128, TD], F32)
          nc.vector.tensor_copy(iota_f[:], iota_i[:])

          W1 = cpool.tile([128, HID], F32)
          nc.sync.dma_start(out=W1[:], in_=W1p[:, :])
          W2 = cpool.tile([128, OUT], F32)
          nc.sync.dma_start(out=W2[:], in_=W2p_[:, :])
          W2f = cpool.tile([128, OUT], F32)  # a1-folded, filled post-AR1
          Wc2 = cpool.tile([128, OUT], F32)
          nc.sync.dma_start(out=Wc2[:], in_=Wc2p_[:, :])
          Wc2f = cpool.tile([128, OUT], F32)

          def col(param, n=128, off=0):
              t = cpool.tile([n, 1], F32, tag=f"col_{param.name}_{off}")
              nc.sync.dma_start(out=t[:], in_=param[off:off + n, None])
              return t

          b1c = col(b1p)
          g1c = col(g1p)
          be1c = col(be1p)
          b2c = [col(b2p, off=h * 128) for h in range(HH)]
          g2c = [col(g2p, off=h * 128) for h in range(HH)]
          be2c = [col(be2p, off=h * 128) for h in range(HH)]
          bc1c = col(bc1p)
          gc1c = col(gc1p)
          bec1c = col(bec1p)
          bc2c = [col(bc2p_, off=h * 128) for h in range(HH)]

          # stats accumulators (per-tile columns)
          st1_sum = stats_p.tile([128, NT], F32)
          st1_sq = stats_p.tile([128, NT], F32)
          st2_sum = stats_p.tile([128, HH * NT], F32)
          st2_sq = stats_p.tile([128, HH * NT], F32)
          stc_sum = stats_p.tile([128, 2], F32)
          stc_sq = stats_p.tile([128, 2], F32)

          # ============ LAYER 1 ============
          def agg_layer(idx_param, table, self_table, post, tag):
              """Aggregation loop per dst tile:
              self-loop contribution via scaled-identity transpose matmuls,
              edges via per-chunk indirect gathers + one-hot matmuls."""
              for t in range(NT):
                  idx_t = meta_p.tile([128, C], I32, tag=f"idx{tag}")
                  nc.sync.dma_start(out=idx_t[:], in_=idx_param[t, :, :])
                  dstl_t = meta_p.tile([128, C], F32, tag=f"dst{tag}")
                  nc.sync.dma_start(out=dstl_t[:], in_=dstl[t, :, :])
                  scale_t = meta_p.tile([128, C], F32, tag=f"scl{tag}")
                  nc.sync.dma_start(out=scale_t[:], in_=scale[t, :, :])
                  d2_t = meta_p.tile([128, 2], F32, tag=f"d2{tag}")
                  nc.sync.dma_start(out=d2_t[:], in_=dinv2[t, :, :])
                  zTf = psMM.tile([128, 512], F32, tag="mm", space="PSUM")
                  zT = zTf[:, :TD]
                  ptrs = []
                  for h in range(TD // 128):
                      sf = rows_p.tile([128, CH], F32, tag=f"sf{tag}")
                      nc.sync.dma_start(
                          out=sf[:],
                          in_=self_table[t * TD + h * 128: t * TD + (h + 1) * 128, :])
                      DIAG = s_p.tile([128, 128], F32, tag=f"dg{tag}")
                      nc.vector.tensor_scalar(
                          out=DIAG[:], in0=ident[:],
                          scalar1=d2_t[:, h:h + 1], scalar2=None, op0=ALU.mult)
                      ptr = psTR.tile([128, 128], F32, tag="tr", space="PSUM")
                      nc.tensor.matmul(ptr[:], lhsT=sf[:], rhs=DIAG[:],
                                       start=True, stop=True)
                      ptrs.append(ptr)
                  for j in range(C):
                      rows = rows_p.tile([128, CH], F32, tag=f"rows{tag}")
                      nc.gpsimd.indirect_dma_start(
                          out=rows[:],
                          out_offset=None,
                          in_=table[:, :],
                          in_offset=bass.IndirectOffsetOnAxis(ap=idx_t[:, j:j + 1], axis=0),
                      )
                      S = s_p.tile([128, TD], F32, tag=f"S{tag}")
                      nc.vector.tensor_scalar(
                          out=S[:], in0=iota_f[:],
                          scalar1=dstl_t[:, j:j + 1], scalar2=scale_t[:, j:j + 1],
                          op0=ALU.is_equal, op1=ALU.mult,
                      )
                      nc.tensor.matmul(
                          zT, lhsT=rows[:], rhs=S[:],
                          start=(j == 0), stop=(j == C - 1),
                      )
                  zs = work_p.tile([128, TD], F32, tag=f"zs{tag}")
                  nc.vector.tensor_copy(zs[:], zT)
                  for h in range(TD // 128):
                      nc.vector.tensor_tensor(
                          out=zs[:, h * 128:(h + 1) * 128],
                          in0=zs[:, h * 128:(h + 1) * 128], in1=ptrs[h][:],
                          op=ALU.add)
                  post(t, zs)

          def l1_post(t, zs):
              uT = psMM.tile([128, 512], F32, tag="mm", space="PSUM")
              nc.tensor.matmul(uT[:, :TD], lhsT=W1[:], rhs=zs[:], start=True, stop=True)
              u1t = work_p.tile([128, TD], F32, tag="u1t")
              nc.scalar.activation(u1t[:], uT[:, :TD], AF.Relu, bias=b1c[:],
                                   accum_out=st1_sum[:, t:t + 1])
              sq = work_p.tile([128, TD], F32, tag="sq1")
              nc.scalar.activation(sq[:], u1t[:], AF.Square,
                                   accum_out=st1_sq[:, t:t + 1])
              for h in range(TD // 128):
                  pt = psTR.tile([128, 128], F32, tag="tr", space="PSUM")
                  nc.tensor.transpose(pt[:], u1t[:, h * 128:(h + 1) * 128], ident[:])
                  rt = tr_p.tile([128, 128], F32, tag="l1rt")
                  nc.vector.tensor_copy(rt[:], pt[:])
                  nc.sync.dma_start(
                      out=u1_shard[t * TD + h * 128: t * TD + (h + 1) * 128, :],
                      in_=rt[:])

          parts = cfg.get("parts", "all")
          agg_layer(idx_l1, x_full, x_shard, l1_post, "a")
          if parts == "l1":
              for q in range(17):
                  tt = work_p.tile([128, 128], F32, tag="dbgcopy")
                  nc.sync.dma_start(out=tt[:], in_=u1_shard[q * 128:(q + 1) * 128, :])
                  nc.sync.dma_start(out=out[q * 128:(q + 1) * 128, :128], in_=tt[:])

          if parts == "l1":
              raise _PartDone()
          # stats -> DRAM for AR1 (BN1 + BNc later in same buffer; BNc cols 2:4)
          st1_red = stats_p.tile([128, 4], F32)
          nc.vector.tensor_reduce(st1_red[:, 0:1], st1_sum[:, :NT], mybir.AxisListType.X, ALU.add)
          nc.vector.tensor_reduce(st1_red[:, 1:2], st1_sq[:, :NT], mybir.AxisListType.X, ALU.add)

          # ============ CELL pass 1 (tanh + stats) ============
          # transpose gexpr chunks into cT strips [128hid, BPC]
          KCs = cfg["KC_CELL"]
          n_bh = BPC // 512
          cT = []  # per batch-half SBUF [128, 512]
          Wc1t = []
          koff = 0
          for ki, kk in enumerate(KCs):
              w = cpool.tile([128, HID], F32, tag=f"wc1_{ki}")
              nc.sync.dma_start(out=w[:kk, :], in_=Wc1p[koff:koff + kk, :])
              Wc1t.append(w)
              koff += kk
          for bh in range(n_bh):
              pc = psMM.tile([128, 512], F32, tag="mm", space="PSUM")
              koff = 0
              for ki, kk in enumerate(KCs):
                  # build rhs [kk, 512]: transpose 4 blocks of gexpr [128b, kk]
                  rhs = cell_p.tile([128, 512], F32, tag="cellrhs")
                  for bb in range(4):
                      gt = cell_p.tile([128, 128], F32, tag="cellg")
                      nc.sync.dma_start(
                          out=gt[:, :kk],
                          in_=gexpr[bh * 512 + bb * 128:bh * 512 + (bb + 1) * 128,
                                    koff:koff + kk])
                      pt = psTR.tile([128, 128], F32, tag="tr", space="PSUM")
                      nc.tensor.transpose(pt[:kk, :], gt[:, :kk], ident[:])
                      nc.vector.tensor_copy(rhs[:kk, bb * 128:(bb + 1) * 128], pt[:kk, :])
                  nc.tensor.matmul(pc[:], lhsT=Wc1t[ki][:kk, :], rhs=rhs[:kk, :],
                                   start=(ki == 0), stop=(ki == len(KCs) - 1))
                  koff += kk
              ct = cell_p.tile([128, 512], F32, tag="cellct")
              nc.scalar.activation(ct[:], pc[:], AF.Tanh, bias=bc1c[:],
                                   accum_out=stc_sum[:, bh:bh + 1])
              csq = cell_p.tile([128, 512], F32, tag="cellsq")
              nc.scalar.activation(csq[:], ct[:], AF.Square,
                                   accum_out=stc_sq[:, bh:bh + 1])
              cT.append(ct)
          nc.vector.tensor_reduce(st1_red[:, 2:3], stc_sum[:, :], mybir.AxisListType.X, ALU.add)
          nc.vector.tensor_reduce(st1_red[:, 3:4], stc_sq[:, :], mybir.AxisListType.X, ALU.add)
          st1_sb = stats_p.tile([128, 4], F32)
          nc.sync.dma_start(out=st1_in[:, :], in_=st1_red[:])

          # AR1 + AG (issue order matters: small first)
          tc.strict_bb_all_engine_barrier()
          nc.gpsimd.collective_compute(
              "AllReduce", ALU.add, replica_groups=rg,
              ins=[st1_in[:]], outs=[st1_out[:]])
          nc.gpsimd.collective_compute(
              "AllGather", ALU.bypass, replica_groups=rg,
              ins=[u1_shard[:]], outs=[u1_full[:]])
          nc.sync.dma_start(out=st1_sb[:], in_=st1_out[:, :])

          # ---- post-AR1 affine computation (BN1 + BNc) ----
          def bn_affine(sum_c, sq_c, gc, bec, bias_relu_col, n_real, n_pad, pfx):
              """returns (a_col, c_col) tiles [128,1]. bias_relu_col: column whose
              relu'ed value times n_pad is subtracted from stats (None to skip)."""
              a_c = cpool.tile([128, 1], F32, tag=f"{pfx}_a")
              c_c = cpool.tile([128, 1], F32, tag=f"{pfx}_c")
              m_c = cpool.tile([128, 1], F32, tag=f"{pfx}_m")
              q_c = cpool.tile([128, 1], F32, tag=f"{pfx}_q")
              t1 = cpool.tile([128, 1], F32, tag=f"{pfx}_t1")
              if bias_relu_col is not None:
                  rb = cpool.tile([128, 1], F32, tag=f"{pfx}_rb")
                  nc.scalar.activation(rb[:], bias_relu_col[:], AF.Relu)
                  rb2 = cpool.tile([128, 1], F32, tag=f"{pfx}_rb2")
                  nc.scalar.activation(rb2[:], rb[:], AF.Square)
                  # m = (sum - npad*rb)/nreal
                  nc.vector.tensor_scalar(out=m_c[:], in0=rb[:], scalar1=float(-n_pad),
                                          scalar2=None, op0=ALU.mult)
                  nc.vector.tensor_tensor(out=m_c[:], in0=m_c[:], in1=sum_c, op=ALU.add)
                  nc.vector.tensor_scalar(out=m_c[:], in0=m_c[:], scalar1=1.0 / n_real,
                                          scalar2=None, op0=ALU.mult)
                  nc.vector.tensor_scalar(out=q_c[:], in0=rb2[:], scalar1=float(-n_pad),
                                          scalar2=None, op0=ALU.mult)
                  nc.vector.tensor_tensor(out=q_c[:], in0=q_c[:], in1=sq_c, op=ALU.add)
                  nc.vector.tensor_scalar(out=q_c[:], in0=q_c[:], scalar1=1.0 / n_real,
                                          scalar2=None, op0=ALU.mult)
              else:
                  nc.vector.tensor_scalar(out=m_c[:], in0=sum_c, scalar1=1.0 / n_real,
                                          scalar2=None, op0=ALU.mult)
                  nc.vector.tensor_scalar(out=q_c[:], in0=sq_c, scalar1=1.0 / n_real,
                                          scalar2=None, op0=ALU.mult)
              # v = q - m^2 ; a = g / sqrt(v+eps) ; c = be - m*a
              nc.scalar.activation(t1[:], m_c[:], AF.Square)
              nc.vector.tensor_tensor(out=t1[:], in0=q_c[:], in1=t1[:], op=ALU.subtract)
              nc.vector.tensor_scalar(out=t1[:], in0=t1[:], scalar1=float(EPS),
                                      scalar2=None, op0=ALU.add)
              nc.vector.reciprocal(t1[:], t1[:])
              nc.scalar.activation(t1[:], t1[:], AF.Sqrt)
              nc.vector.tensor_tensor(out=a_c[:], in0=gc[:], in1=t1[:], op=ALU.mult)
              nc.vector.tensor_tensor(out=c_c[:], in0=m_c[:], in1=a_c[:], op=ALU.mult)
              nc.vector.tensor_tensor(out=c_c[:], in0=bec[:], in1=c_c[:], op=ALU.subtract)
              return a_c, c_c

          a1c, c1c = bn_affine(st1_sb[:, 0:1], st1_sb[:, 1:2], g1c, be1c, b1c,
                               NREAL, NPAD, "bn1")
          acc_, ccc_ = bn_affine(st1_sb[:, 2:3], st1_sb[:, 3:4], gc1c, bec1c, None,
                                 cfg["N_GRAPHS"], 0, "bnc")

          # W2' = a1 * W2 ; r2 rows [1,128] per half via matmul
          nc.vector.tensor_scalar(out=W2f[:], in0=W2[:], scalar1=a1c[:],
                                  scalar2=None, op0=ALU.mult)
          r2 = []
          for h in range(HH):
              pr = psTR.tile([128, 128], F32, tag="tr", space="PSUM")
              nc.tensor.matmul(pr[:1, :], lhsT=c1c[:], rhs=W2[:, h * 128:(h + 1) * 128],
                               start=True, stop=True)
              rr = cpool.tile([1, 128], F32, tag=f"r2_{h}")
              nc.vector.tensor_copy(rr[:], pr[:1, :])
              r2.append(rr)

          # cell finish: Wc2' = ac*Wc2; bias cols = (ccf @ Wc2half).T + bc2half
          nc.vector.tensor_scalar(out=Wc2f[:], in0=Wc2[:], scalar1=acc_[:],
                                  scalar2=None, op0=ALU.mult)
          bc2f = []
          for h in range(HH):
              pb = psTR.tile([128, 128], F32, tag="tr", space="PSUM")
              nc.tensor.matmul(pb[:, :1], lhsT=Wc2[:, h * 128:(h + 1) * 128], rhs=ccc_[:],
                               start=True, stop=True)
              bb_ = cpool.tile([128, 1], F32, tag=f"bc2f_{h}")
              nc.vector.tensor_tensor(out=bb_[:], in0=pb[:, :1], in1=bc2c[h][:], op=ALU.add)
              bc2f.append(bb_)
          for bh in range(n_bh):
              for h in range(HH):
                  px = psMM.tile([128, 512], F32, tag="mm", space="PSUM")
                  nc.tensor.matmul(px[:], lhsT=Wc2f[:, h * 128:(h + 1) * 128],
                                   rhs=cT[bh][:], start=True, stop=True)
                  xc = cell_p.tile([128, 512], F32, tag="cellxc")
                  nc.scalar.activation(xc[:], px[:], AF.Relu, bias=bc2f[h][:])
                  for bb in range(4):
                      pt = psTR.tile([128, 128], F32, tag="tr", space="PSUM")
                      nc.tensor.transpose(pt[:], xc[:, bb * 128:(bb + 1) * 128], ident[:])
                      rt = tr_p.tile([128, 128], F32, tag="cellort")
                      nc.vector.tensor_copy(rt[:], pt[:])
                      nc.sync.dma_start(
                          out=out[G_OUT + bh * 512 + bb * 128:
                                  G_OUT + bh * 512 + (bb + 1) * 128,
                                  h * 128:(h + 1) * 128],
                          in_=rt[:])

          if parts == "l1c":
              raise _PartDone()
          # ============ LAYER 2 ============
          def l2_post(t, zs):
              s_t = meta_p.tile([1, TD], F32, tag="s_t")
              nc.sync.dma_start(out=s_t[:], in_=s_rows[t, None, :])
              for h in range(HH):
                  uT = psMM.tile([128, 512], F32, tag="mm", space="PSUM")
                  nc.tensor.matmul(uT[:, :TD], lhsT=W2f[:, h * 128:(h + 1) * 128],
                                   rhs=zs[:], start=True, stop=False)
                  nc.tensor.matmul(uT[:, :TD], lhsT=r2[h][:],
                                   rhs=s_t[:], start=False, stop=True)
                  u2t = work_p.tile([128, TD], F32, tag="u2t")
                  nc.scalar.activation(u2t[:], uT[:, :TD], AF.Relu, bias=b2c[h][:],
                                       accum_out=st2_sum[:, t * HH + h:t * HH + h + 1])
                  sq = work_p.tile([128, TD], F32, tag="sq2")
                  nc.scalar.activation(sq[:], u2t[:], AF.Square,
                                       accum_out=st2_sq[:, t * HH + h:t * HH + h + 1])
                  for q in range(TD // 128):
                      pt = psTR.tile([128, 128], F32, tag="tr", space="PSUM")
                      nc.tensor.transpose(pt[:], u2t[:, q * 128:(q + 1) * 128], ident[:])
                      rt = tr_p.tile([128, 128], F16, tag="l2rt")
                      nc.vector.tensor_copy(rt[:], pt[:])
                      nc.sync.dma_start(
                          out=u2pool[t * TD + q * 128: t * TD + (q + 1) * 128,
                                     h * 128:(h + 1) * 128],
                          in_=rt[:])

          agg_layer(idx_l2, u1_full, u1_shard, l2_post, "b")

          if parts == "nol2stats":
              raise _PartDone()
          st2_red = stats_p.tile([128, 4], F32)
          # halves interleaved: cols t*2+h -> reduce per h via strided AP
          for h in range(HH):
              nc.vector.tensor_reduce(
                  st2_red[:, 2 * h:2 * h + 1],
                  st2_sum[:].rearrange("p (t h) -> p t h", h=HH)[:, :, h],
                  mybir.AxisListType.X, ALU.add)
              nc.vector.tensor_reduce(
                  st2_red[:, 2 * h + 1:2 * h + 2],
                  st2_sq[:].rearrange("p (t h) -> p t h", h=HH)[:, :, h],
                  mybir.AxisListType.X, ALU.add)
          nc.sync.dma_start(out=st2_in[:, :], in_=st2_red[:])
          tc.strict_bb_all_engine_barrier()
          nc.gpsimd.collective_compute(
              "AllReduce", ALU.add, replica_groups=rg,
              ins=[st2_in[:]], outs=[st2_out[:]])
          st2_sb = stats_p.tile([128, 4], F32)
          nc.sync.dma_start(out=st2_sb[:], in_=st2_out[:, :])

          a2c, c2c = [], []
          for h in range(HH):
              a_, c_ = bn_affine(st2_sb[:, 2 * h:2 * h + 1], st2_sb[:, 2 * h + 1:2 * h + 2],
                                 g2c[h], be2c[h], b2c[h], NREAL, NPAD, f"bn2_{h}")
              a2c.append(a_)
              c2c.append(c_)

          if parts == "nopool":
              raise _PartDone()
          # ============ POOL ============
          pooled = pool_p.tile([128, HH, G_PAD], F32)
          for (n_idx, n_g, L, col_off, out_off) in cfg["pool_ops"]:
              pidx = pool_p.tile([128, n_idx // 16], I16, tag="pidx")
              nc.sync.dma_start(out=pidx[:], in_=pool_idx[:, col_off:col_off + n_idx // 16])
              gath = pool_p.tile([128, HH, n_idx], F16, tag="pgath")
              nc.gpsimd.dma_gather(
                  gath[:], u2pool[:, :], pidx[:], n_idx, n_idx,
                  elem_size=OUT, transpose=True, single_packet=False)
              nc.vector.tensor_reduce(
                  pooled[:, :, out_off:out_off + n_g],
                  gath[:].rearrange("p h (g l) -> p h g l", g=n_g),
                  mybir.AxisListType.X, ALU.max)
          # BN2 affine on pooled
          for h in range(HH):
              nc.vector.tensor_scalar(
                  out=pooled[:, h, :], in0=pooled[:, h, :],
                  scalar1=a2c[h][:], scalar2=c2c[h][:], op0=ALU.mult, op1=ALU.add)
          # transpose to out rows
          for h in range(HH):
              for q in range(G_OUT // 128):
                  n_here = min(128, G_PAD - q * 128)
                  if n_here <= 0:
                      break
                  pt = psTR.tile([128, 128], F32, tag="tr", space="PSUM")
                  nc.tensor.transpose(pt[:n_here, :], pooled[:, h, q * 128:q * 128 + n_here],
                                      ident[:])
                  rt = tr_p.tile([128, 128], F32, tag="poolrt")
                  nc.vector.tensor_copy(rt[:n_here, :], pt[:n_here, :])
                  nc.sync.dma_start(
                      out=out[q * 128:q * 128 + n_here, h * 128:(h + 1) * 128],
                      in_=rt[:n_here, :])

      except _PartDone:
          pass
    nc.compile()
    return nc


_BUILD_CACHE = {}


def _run(inputs, trace=False):
    plan = build_plan(inputs["drug_adj"], inputs["ibatch"])
    cfg, in_maps, assemble = make_cfg_inputs(plan, inputs)
    key = (cfg["NT"], cfg["C"], cfg["G_PAD"], tuple(map(tuple, cfg["pool_ops"])))
    if key not in _BUILD_CACHE:
        _BUILD_CACHE[key] = build_kernel(cfg)
    nc = _BUILD_CACHE[key]
    res = run_bass_kernel_spmd(nc, in_maps, core_ids=list(range(8)), trace=trace)
    x_drug, x_cell = assemble(res.results)
    return (x_drug, x_cell), res


def kernel(**inputs):
    inputs = {k: np.asarray(v) for k, v in inputs.items()}
    (x_drug, x_cell), _ = _run(inputs, trace=False)
    return x_drug, x_cell



# revision 7
# speedup vs baseline: 2.7314x; 2.7314x over previous
"""Self-contained distributed Bass kernel for nn_Atom_Gloal_37958920962359.

Two-layer GCN (PyG GCNConv semantics) + batchnorm + global max pool over
200k nodes / 800k edges / 8192 graphs, plus a cell-line MLP branch, running
SPMD on 8 TRN2 NeuronCores.

v2 strategy (fp16 + host-built scatter matrices + merged gathers):
- graph-aligned node/edge shards per core; GCN as z = [Dinv (A+I) Dinv] x,
  u = z @ W + b so aggregation commutes with the weight matmul.
- aggregation per 256-dst tile: one fp16 matmul per 128-slot chunk with the
  one-hot scatter matrix S (incl. self-loop chunks) PRECOMPUTED ON HOST and
  streamed from DRAM; gathered source rows fetched with ONE indirect DMA per
  4-tile group (SWDGE fixed cost amortized).
- all matmul operands fp16 (PSUM accumulates fp32); tables (x, u1, u2pool)
  in fp16 so gather/AllGather bytes halve.
- BN affines folded into downstream matmuls (stats via ACT accum_out +
  small AllReduce + pad correction); layer-1 table replicated via fp16
  AllGather; pool via fp16 dma_gather with graphs bucketed by padded node
  count into uniform segmented max-reduces.
"""
import sys
sys.path.insert(0, "/opt/trn_rl_repo")

import numpy as np
from contextlib import ExitStack

import concourse.bass as bass
import concourse.bacc as bacc
import concourse.mybir as mybir
import concourse.tile as tile
from concourse.masks import make_identity
from concourse.bass_utils import run_bass_kernel_spmd


N_NODES = 200000
N_EDGES = 800000
N_GRAPHS = 8192
DIM_DRUG = 128
HID = 128
OUT = 256
DIM_CELL = 954
EPS = 1e-5
N_CORES = 8
TILE_DST = 256      # dst nodes per psum tile
CHUNK = 128         # slots per matmul chunk (K dim)
GPC = N_GRAPHS // N_CORES  # graphs per core
POOL_SUB = 8        # pool group size (level-1 max granularity)
GMAX = 4            # tiles per gather group
NSELF = 2           # self-loop chunks per tile (TILE_DST // CHUNK)


def build_plan(drug_adj, ibatch):
    """All index preprocessing + host-built fp16 scatter matrices."""
    ibatch = np.asarray(ibatch)
    src_all = np.asarray(drug_adj[0]).astype(np.int64)
    dst_all = np.asarray(drug_adj[1]).astype(np.int64)

    node_start = np.searchsorted(ibatch, np.arange(N_CORES + 1) * GPC).astype(np.int64)
    nodes_c = np.diff(node_start)

    deg = np.bincount(dst_all, minlength=N_NODES).astype(np.int64) + 1
    dinv = 1.0 / np.sqrt(deg.astype(np.float64))

    owner_of_node = np.searchsorted(node_start, np.arange(N_NODES), side="right") - 1
    edge_owner = owner_of_node[dst_all]

    NT = int(np.max(np.ceil(nodes_c / TILE_DST)).astype(int))
    SH = NT * TILE_DST
    NGRP = (NT + GMAX - 1) // GMAX

    cores = []
    maxC = 0
    per_core_slots = []
    for c in range(N_CORES):
        lo, hi = node_start[c], node_start[c + 1]
        m = edge_owner == c
        s_c = src_all[m]
        d_c = dst_all[m]
        order = np.argsort(d_c, kind="stable")
        s_c, d_c = s_c[order], d_c[order]
        dloc = d_c - lo
        tile_of_slot = dloc // TILE_DST
        cnt = np.bincount(tile_of_slot, minlength=NT)
        maxC = max(maxC, int(np.ceil(cnt / CHUNK).max()))
        per_core_slots.append((s_c, dloc, cnt, lo, hi))

    C = maxC
    spt = C * CHUNK  # slots per tile
    KCH = NSELF + C  # chunks per tile incl self

    for c in range(N_CORES):
        s_c, dloc, cnt, lo, hi = per_core_slots[c]
        nreal = int(hi - lo)
        total_slots = NT * spt
        src_slots = np.zeros(total_slots, dtype=np.int64)
        dstl_slots = np.full(total_slots, -1, dtype=np.int64)
        scale_slots = np.zeros(total_slots, dtype=np.float32)
        tile_offsets = np.zeros(NT + 1, dtype=np.int64)
        tile_offsets[1:] = np.cumsum(cnt)
        for t in range(NT):
            a, b = tile_offsets[t], tile_offsets[t + 1]
            n = b - a
            base = t * spt
            src_slots[base:base + n] = s_c[a:b]
            dstl_slots[base:base + n] = dloc[a:b] - t * TILE_DST
            scale_slots[base:base + n] = (dinv[s_c[a:b]] * dinv[dloc[a:b] + lo]).astype(np.float32)

        # d2 (self-loop scale) and s-vector per padded shard row
        d2 = np.zeros(SH, dtype=np.float32)
        d2[:nreal] = (dinv[lo:hi] ** 2).astype(np.float32)
        s_vec = np.zeros(SH, dtype=np.float64)
        vmask = dstl_slots >= 0
        np.add.at(s_vec, (np.arange(total_slots) // spt) * TILE_DST + np.where(vmask, dstl_slots, 0),
                  np.where(vmask, scale_slots.astype(np.float64), 0.0))
        s_vec += d2.astype(np.float64)

        # scatter matrices S [NT, KCH, 128, TD] fp16 (self chunks first)
        S = np.zeros((NT, KCH, CHUNK, TILE_DST), dtype=np.float16)
        idxs = np.nonzero(vmask)[0]
        tt = idxs // spt
        rr = idxs % spt
        jj = rr // CHUNK
        pp = rr % CHUNK
        S[tt, NSELF + jj, pp, dstl_slots[idxs]] = scale_slots[idxs].astype(np.float16)
        rowl = np.arange(SH)
        tts = rowl // TILE_DST
        rrs = rowl % TILE_DST
        S[tts, rrs // CHUNK, rrs % CHUNK, rrs] = d2.astype(np.float16)
        pad = NGRP * GMAX - NT
        if pad:
            S = np.concatenate([S, np.zeros((pad, KCH, CHUNK, TILE_DST), np.float16)], 0)
        Sw = np.ascontiguousarray(
            S.reshape(NGRP, GMAX, KCH, CHUNK, TILE_DST)
            .transpose(0, 3, 1, 2, 4)
            .reshape(NGRP, CHUNK, GMAX * KCH * TILE_DST))

        # gather index packs [128, NT*C]
        def pack(arr):
            return np.ascontiguousarray(
                arr.reshape(NT, C, CHUNK).transpose(2, 0, 1).reshape(CHUNK, NT * C)
            ).astype(np.int32)

        idx_l1 = pack(src_slots)
        own = owner_of_node[src_slots]
        l2_rows = own * SH + (src_slots - node_start[own])
        idx_l2 = pack(l2_rows)

        cores.append(dict(
            lo=int(lo), hi=int(hi), nreal=nreal,
            idx_l1=idx_l1, idx_l2=idx_l2, Sw=Sw,
            s_flat=s_vec.astype(np.float16),
        ))

    # ---- pool layout (same as baseline) ----
    gcnt = np.bincount(ibatch, minlength=N_GRAPHS)
    maxg = int(gcnt.max())
    KMAX = int(np.ceil(maxg / POOL_SUB))
    pool = dict(KMAX=KMAX, maxg=maxg)
    bucket_counts = np.zeros((N_CORES, KMAX + 1), dtype=np.int64)
    for c in range(N_CORES):
        g0 = c * GPC
        kg = np.ceil(gcnt[g0:g0 + GPC] / POOL_SUB).astype(np.int64)
        for kk in range(1, KMAX + 1):
            bucket_counts[c, kk] = int((kg == kk).sum())
    pool["caps"] = bucket_counts.max(axis=0)

    return dict(cores=cores, NT=NT, SH=SH, C=C, KCH=KCH, NGRP=NGRP,
                node_start=node_start, dinv=dinv, pool=pool,
                n_pad_nodes=int(N_CORES * SH - N_NODES))


def make_pool_ops(caps, KMAX, max_op_idx=1024):
    from math import gcd
    pool_ops = []
    caps_pad = np.zeros(KMAX + 1, dtype=np.int64)
    col_off = 0
    out_off = 0
    for kk in range(1, KMAX + 1):
        if caps[kk] == 0:
            continue
        L = POOL_SUB * kk
        m = 128 // gcd(L, 128)
        cap = int(np.ceil(caps[kk] / m) * m)
        caps_pad[kk] = cap
        gpo = max(m, (max_op_idx // L) // m * m)
        g = 0
        while g < cap:
            ng = int(min(gpo, cap - g))
            n_idx = ng * L
            assert n_idx % 128 == 0
            pool_ops.append((n_idx, ng, L, col_off, out_off))
            col_off += n_idx // 16
            out_off += ng
            g += ng
    return pool_ops, caps_pad, int(out_off), int(col_off)


def build_pool_core(plan, c, ibatch, caps_pad, KMAX, total_cols):
    node_start = plan["node_start"]
    gcnt = np.bincount(ibatch, minlength=N_GRAPHS)
    g0 = c * GPC
    lo, hi = node_start[c], node_start[c + 1]
    kg = np.ceil(gcnt[g0:g0 + GPC] / POOL_SUB).astype(np.int64)
    local_ib = np.asarray(ibatch)[lo:hi]
    gs = np.searchsorted(local_ib, np.arange(g0, g0 + GPC + 1))
    slot_idx = []
    graph_order = []
    for kk in range(1, KMAX + 1):
        if caps_pad[kk] == 0:
            continue
        L = kk * POOL_SUB
        gsel = np.where(kg == kk)[0]
        for g in gsel:
            a, b = gs[g], gs[g + 1]
            n = b - a
            ids = np.empty(L, dtype=np.int64)
            ids[:n] = np.arange(a, b)
            ids[n:] = a
            slot_idx.append(ids)
            graph_order.append(g)
        for _ in range(int(caps_pad[kk] - len(gsel))):
            slot_idx.append(np.zeros(L, dtype=np.int64))
            graph_order.append(-1)
    flat = np.concatenate(slot_idx).astype(np.int16)
    assert len(flat) == total_cols * 16
    wrapped = np.empty((128, total_cols), dtype=np.int16)
    for p in range(128):
        wrapped[p, :] = flat[np.arange(total_cols) * 16 + (p % 16)]
    return wrapped, np.array(graph_order, dtype=np.int64)


def make_cfg_inputs(plan, inputs):
    ibatch = np.asarray(inputs["ibatch"])
    caps, KMAX = plan["pool"]["caps"], plan["pool"]["KMAX"]
    pool_ops, caps_pad, G_PAD, POOL_COLS = make_pool_ops(caps, KMAX)
    G_OUT = int(np.ceil(G_PAD / 128) * 128)
    KC_CELL = [128] * (DIM_CELL // 128) + ([DIM_CELL % 128] if DIM_CELL % 128 else [])

    cfg = dict(
        NT=plan["NT"], C=plan["C"], SH=plan["SH"], KCH=plan["KCH"],
        NGRP=plan["NGRP"], n_pad_nodes=plan["n_pad_nodes"],
        pool_ops=pool_ops, POOL_COLS=POOL_COLS,
        G_PAD=G_PAD, G_OUT=G_OUT, KC_CELL=KC_CELL,
    )

    x16 = np.asarray(inputs["drug_feature"], dtype=np.float16)
    gex = np.asarray(inputs["gexpr_data"], dtype=np.float32)
    wf32 = {k: np.ascontiguousarray(np.asarray(inputs[k], dtype=np.float32))
            for k in ("W2", "b1", "g1", "be1", "b2", "g2", "be2",
                      "bc1", "gc1", "bec1", "Wc2", "bc2")}
    W1_16 = np.ascontiguousarray(np.asarray(inputs["W1"], dtype=np.float16))
    Wc1_16 = np.ascontiguousarray(np.asarray(inputs["Wc1"], dtype=np.float16))

    in_maps = []
    orders = []
    for c in range(N_CORES):
        core = plan["cores"][c]
        pidx, gorder = build_pool_core(plan, c, ibatch, caps_pad, KMAX, POOL_COLS)
        orders.append(gorder)
        xs = np.zeros((plan["SH"], DIM_DRUG), dtype=np.float16)
        xs[:core["nreal"]] = x16[core["lo"]:core["hi"]]
        gexT = np.ascontiguousarray(gex[c * GPC:(c + 1) * GPC].T.astype(np.float16))
        m = dict(
            x_full=x16, x_shard=xs, Sw=core["Sw"],
            idx_l1=core["idx_l1"], idx_l2=core["idx_l2"],
            s_flat=core["s_flat"].reshape(1, -1),
            pool_idx=pidx, gexprT=gexT, W1=W1_16, Wc1=Wc1_16,
            **wf32,
        )
        in_maps.append(m)

    def assemble(results):
        x_drug = np.full((N_GRAPHS, OUT), -np.inf, dtype=np.float32)
        x_cell = np.empty((N_GRAPHS, OUT), dtype=np.float32)
        for c in range(N_CORES):
            o = np.asarray(results[c]["out"])
            gorder = orders[c]
            rows = np.nonzero(gorder >= 0)[0]
            x_drug[c * GPC + gorder[rows]] = o[rows]
            x_cell[c * GPC:(c + 1) * GPC] = o[G_OUT:G_OUT + GPC]
        return x_drug, x_cell

    return cfg, in_maps, assemble


class _PartDone(Exception):
    pass


F32 = mybir.dt.float32
F16 = mybir.dt.float16
I32 = mybir.dt.int32
I16 = mybir.dt.int16
AF = mybir.ActivationFunctionType
ALU = mybir.AluOpType


def build_kernel(cfg):
    NT, C, SH, KCH, NGRP = cfg["NT"], cfg["C"], cfg["SH"], cfg["KCH"], cfg["NGRP"]
    TD, CH = TILE_DST, CHUNK
    NPAD = cfg["n_pad_nodes"]
    G_PAD, G_OUT = cfg["G_PAD"], cfg["G_OUT"]
    HH = OUT // 128
    NREAL = N_NODES
    groups = [(t0, min(t0 + GMAX, NT)) for t0 in range(0, NT, GMAX)]

    nc = bacc.Bacc(None, num_devices=N_CORES)

    # ---------------- parameters ----------------
    x_full = nc.declare_dram_parameter("x_full", [N_NODES, DIM_DRUG], F16, isOutput=False)
    x_shard = nc.declare_dram_parameter("x_shard", [SH, DIM_DRUG], F16, isOutput=False)
    Swp = nc.declare_dram_parameter("Sw", [NGRP, CH, GMAX * KCH * TD], F16, isOutput=False)
    idx_l1 = nc.declare_dram_parameter("idx_l1", [CH, NT * C], I32, isOutput=False)
    idx_l2 = nc.declare_dram_parameter("idx_l2", [CH, NT * C], I32, isOutput=False)
    s_flat = nc.declare_dram_parameter("s_flat", [1, SH], F16, isOutput=False)
    pool_idx = nc.declare_dram_parameter("pool_idx", [CH, cfg["POOL_COLS"]], I16, isOutput=False)
    gexprT = nc.declare_dram_parameter("gexprT", [DIM_CELL, GPC], F16, isOutput=False)
    W1p = nc.declare_dram_parameter("W1", [DIM_DRUG, HID], F16, isOutput=False)
    b1p = nc.declare_dram_parameter("b1", [HID], F32, isOutput=False)
    g1p = nc.declare_dram_parameter("g1", [HID], F32, isOutput=False)
    be1p = nc.declare_dram_parameter("be1", [HID], F32, isOutput=False)
    W2p_ = nc.declare_dram_parameter("W2", [HID, OUT], F32, isOutput=False)
    b2p = nc.declare_dram_parameter("b2", [OUT], F32, isOutput=False)
    g2p = nc.declare_dram_parameter("g2", [OUT], F32, isOutput=False)
    be2p = nc.declare_dram_parameter("be2", [OUT], F32, isOutput=False)
    Wc1p = nc.declare_dram_parameter("Wc1", [DIM_CELL, HID], F16, isOutput=False)
    bc1p = nc.declare_dram_parameter("bc1", [HID], F32, isOutput=False)
    gc1p = nc.declare_dram_parameter("gc1", [HID], F32, isOutput=False)
    bec1p = nc.declare_dram_parameter("bec1", [HID], F32, isOutput=False)
    Wc2p_ = nc.declare_dram_parameter("Wc2", [HID, OUT], F32, isOutput=False)
    bc2p_ = nc.declare_dram_parameter("bc2", [OUT], F32, isOutput=False)
    out = nc.declare_dram_parameter("out", [G_OUT + GPC, OUT], F32, isOutput=True)

    # ---------------- internal DRAM ----------------
    u1_shard = nc.dram_tensor("u1_shard", [SH, HID], F16)
    u1_full = nc.dram_tensor("u1_full", [N_CORES * SH, HID], F16, addr_space="Shared")
    u2pool = nc.dram_tensor("u2pool", [SH, OUT], F16)
    st1_in = nc.dram_tensor("st1_in", [128, 4], F32)
    st1_out = nc.dram_tensor("st1_out", [128, 4], F32, addr_space="Shared")
    st2_in = nc.dram_tensor("st2_in", [128, 4], F32)
    st2_out = nc.dram_tensor("st2_out", [128, 4], F32, addr_space="Shared")

    rg = [list(range(N_CORES))]

    with tile.TileContext(nc) as tc, ExitStack() as ctx:
      try:
        cpool = ctx.enter_context(tc.tile_pool(name="consts", bufs=1))
        rows_p = ctx.enter_context(tc.tile_pool(name="rows", bufs=2))
        sload_p = ctx.enter_context(tc.tile_pool(name="sload", bufs=2))
        sf_p = ctx.enter_context(tc.tile_pool(name="sf", bufs=3))
        work_p = ctx.enter_context(tc.tile_pool(name="work", bufs=4))
        tr_p = ctx.enter_context(tc.tile_pool(name="tr", bufs=3))
        stats_p = ctx.enter_context(tc.tile_pool(name="stats", bufs=1))
        cell_p = ctx.enter_context(tc.tile_pool(name="cell", bufs=2))
        pool_p = ctx.enter_context(tc.tile_pool(name="pool", bufs=2))
        psMM = ctx.enter_context(tc.tile_pool(name="psMM", bufs=4, space="PSUM"))
        psTR = ctx.enter_context(tc.tile_pool(name="psTR", bufs=2, space="PSUM"))
        psTRF = ctx.enter_context(tc.tile_pool(name="psTRF", bufs=1, space="PSUM"))

        # ---------------- constants ----------------
        identf = cpool.tile([128, 128], F32)
        make_identity(nc, identf[:])
        ident16 = cpool.tile([128, 128], F16)
        nc.vector.tensor_copy(ident16[:], identf[:])

        W1w = cpool.tile([128, HID], F16)
        nc.sync.dma_start(out=W1w[:], in_=W1p[:, :])
        W2 = cpool.tile([128, OUT], F32)
        nc.sync.dma_start(out=W2[:], in_=W2p_[:, :])
        W2f = cpool.tile([128, OUT], F16)
        Wc2 = cpool.tile([128, OUT], F32)
        nc.sync.dma_start(out=Wc2[:], in_=Wc2p_[:, :])
        Wc2f = cpool.tile([128, OUT], F16)

        idx1_sb = cpool.tile([128, NT * C], I32)
        nc.sync.dma_start(out=idx1_sb[:], in_=idx_l1[:, :])
        idx2_sb = cpool.tile([128, NT * C], I32)
        nc.sync.dma_start(out=idx2_sb[:], in_=idx_l2[:, :])

        def col(param, n=128, off=0):
            t = cpool.tile([n, 1], F32, tag=f"col_{param.name}_{off}")
            nc.sync.dma_start(out=t[:], in_=param[off:off + n, None])
            return t

        b1c = col(b1p)
        g1c = col(g1p)
        be1c = col(be1p)
        b2c = [col(b2p, off=h * 128) for h in range(HH)]
        g2c = [col(g2p, off=h * 128) for h in range(HH)]
        be2c = [col(be2p, off=h * 128) for h in range(HH)]
        bc1c = col(bc1p)
        gc1c = col(gc1p)
        bec1c = col(bec1p)
        bc2c = [col(bc2p_, off=h * 128) for h in range(HH)]

        st1_sum = stats_p.tile([128, NT], F32)
        st1_sq = stats_p.tile([128, NT], F32)
        st2_sum = stats_p.tile([128, HH * NT], F32)
        st2_sq = stats_p.tile([128, HH * NT], F32)
        stc_sum = stats_p.tile([128, 2], F32)
        stc_sq = stats_p.tile([128, 2], F32)

        # ============ aggregation layer ============
        def agg_layer(idx_sb, table, self_table, post, tag):
            GCOLS = 8  # idx columns per indirect op (1024 descriptors)
            for gi, (t0, t1) in enumerate(groups):
                gl = t1 - t0
                rows = rows_p.tile([128, GMAX * C * CH], F16, tag=f"rows{tag}")
                for s0 in range(0, gl * C, GCOLS):
                    s1 = min(s0 + GCOLS, gl * C)
                    nc.gpsimd.indirect_dma_start(
                        out=rows[:, s0 * CH:s1 * CH],
                        out_offset=None,
                        in_=table[:, :],
                        in_offset=bass.IndirectOffsetOnAxis(
                            ap=idx_sb[:, t0 * C + s0:t0 * C + s1], axis=0),
                    )
                Sg = sload_p.tile([128, GMAX * KCH * TD], F16, tag=f"S{tag}")
                nc.sync.dma_start(out=Sg[:, :gl * KCH * TD],
                                  in_=Swp[gi, :, :gl * KCH * TD])
                for ti in range(gl):
                    t = t0 + ti
                    sf = sf_p.tile([128, NSELF, CH], F16, tag=f"sf{tag}")
                    nc.sync.dma_start(
                        out=sf[:],
                        in_=self_table[t * TD:(t + 1) * TD, :].rearrange(
                            "(h p) f -> p h f", p=CH))
                    zTf = psMM.tile([128, 512], F32, tag="mm", space="PSUM")
                    zT = zTf[:, :TD]
                    base = ti * KCH * TD
                    for h in range(NSELF):
                        nc.tensor.matmul(
                            zT, lhsT=sf[:, h, :],
                            rhs=Sg[:, base + h * TD:base + (h + 1) * TD],
                            start=(h == 0), stop=False)
                    for j in range(C):
                        nc.tensor.matmul(
                            zT, lhsT=rows[:, (ti * C + j) * CH:(ti * C + j + 1) * CH],
                            rhs=Sg[:, base + (NSELF + j) * TD:base + (NSELF + j + 1) * TD],
                            start=False, stop=(j == C - 1))
                    post(t, zT)

        def l1_post(t, zT):
            zs = work_p.tile([128, TD], F16, tag="zs1")
            nc.scalar.copy(zs[:], zT)
            uTf = psMM.tile([128, 512], F32, tag="mm", space="PSUM")
            uT = uTf[:, :TD]
            nc.tensor.matmul(uT, lhsT=W1w[:], rhs=zs[:], start=True, stop=True)
            u1t = work_p.tile([128, TD], F16, tag="u1t")
            nc.scalar.activation(u1t[:], uT, AF.Relu, bias=b1c[:],
                                 accum_out=st1_sum[:, t:t + 1])
            sq = work_p.tile([128, TD], F16, tag="sq1")
            nc.scalar.activation(sq[:], u1t[:], AF.Square,
                                 accum_out=st1_sq[:, t:t + 1])
            rt2 = tr_p.tile([128, NSELF, CH], F16, tag="l1rt")
            for h in range(NSELF):
                pt = psTR.tile([128, CH], F16, tag="tr16", space="PSUM")
                nc.tensor.transpose(pt[:], u1t[:, h * CH:(h + 1) * CH], ident16[:])
                nc.vector.tensor_copy(rt2[:, h, :], pt[:])
            nc.scalar.dma_start(
                out=u1_shard[t * TD:(t + 1) * TD, :].rearrange("(h p) f -> p h f", p=CH),
                in_=rt2[:])

        parts = cfg.get("parts", "all")
        agg_layer(idx1_sb, x_full, x_shard, l1_post, "a")
        if parts == "l1":
            for q in range(17):
                tt = work_p.tile([128, 128], F16, tag="dbgcopy")
                nc.sync.dma_start(out=tt[:], in_=u1_shard[q * 128:(q + 1) * 128, :])
                cv = work_p.tile([128, 128], F32, tag="dbgcv")
                nc.vector.tensor_copy(cv[:], tt[:])
                nc.sync.dma_start(out=out[q * 128:(q + 1) * 128, :128], in_=cv[:])
            raise _PartDone()

        st1_red = stats_p.tile([128, 4], F32)
        nc.vector.tensor_reduce(st1_red[:, 0:1], st1_sum[:, :NT], mybir.AxisListType.X, ALU.add)
        nc.vector.tensor_reduce(st1_red[:, 1:2], st1_sq[:, :NT], mybir.AxisListType.X, ALU.add)

        # ============ CELL pass 1 (tanh + stats) ============
        KCs = cfg["KC_CELL"]
        n_bh = GPC // 512
        Wc1t = []
        koff = 0
        for ki, kk in enumerate(KCs):
            w = cpool.tile([128, HID], F16, tag=f"wc1_{ki}")
            nc.sync.dma_start(out=w[:kk, :], in_=Wc1p[koff:koff + kk, :])
            Wc1t.append(w)
            koff += kk
        cT = []
        for bh in range(n_bh):
            pc = psMM.tile([128, 512], F32, tag="mm", space="PSUM")
            koff = 0
            for ki, kk in enumerate(KCs):
                strip = cell_p.tile([128, 512], F16, tag="strip")
                nc.sync.dma_start(out=strip[:kk, :],
                                  in_=gexprT[koff:koff + kk, bh * 512:(bh + 1) * 512])
                nc.tensor.matmul(pc[:], lhsT=Wc1t[ki][:kk, :], rhs=strip[:kk, :],
                                 start=(ki == 0), stop=(ki == len(KCs) - 1))
                koff += kk
            ct = cell_p.tile([128, 512], F16, tag=f"cellct{bh}")
            nc.scalar.activation(ct[:], pc[:], AF.Tanh, bias=bc1c[:],
                                 accum_out=stc_sum[:, bh:bh + 1])
            csq = cell_p.tile([128, 512], F16, tag="cellsq")
            nc.scalar.activation(csq[:], ct[:], AF.Square,
                                 accum_out=stc_sq[:, bh:bh + 1])
            cT.append(ct)
        nc.vector.tensor_reduce(st1_red[:, 2:3], stc_sum[:, :], mybir.AxisListType.X, ALU.add)
        nc.vector.tensor_reduce(st1_red[:, 3:4], stc_sq[:, :], mybir.AxisListType.X, ALU.add)
        nc.sync.dma_start(out=st1_in[:, :], in_=st1_red[:])

        # AR1 + AG (small first)
        tc.strict_bb_all_engine_barrier()
        nc.gpsimd.collective_compute(
            "AllReduce", ALU.add, replica_groups=rg,
            ins=[st1_in[:]], outs=[st1_out[:]])
        nc.gpsimd.collective_compute(
            "AllGather", ALU.bypass, replica_groups=rg,
            ins=[u1_shard[:]], outs=[u1_full[:]])
        st1_sb = stats_p.tile([128, 4], F32)
        nc.sync.dma_start(out=st1_sb[:], in_=st1_out[:, :])

        def bn_affine(sum_c, sq_c, gc, bec, bias_relu_col, n_real, n_pad, pfx):
            a_c = cpool.tile([128, 1], F32, tag=f"{pfx}_a")
            c_c = cpool.tile([128, 1], F32, tag=f"{pfx}_c")
            m_c = cpool.tile([128, 1], F32, tag=f"{pfx}_m")
            q_c = cpool.tile([128, 1], F32, tag=f"{pfx}_q")
            t1 = cpool.tile([128, 1], F32, tag=f"{pfx}_t1")
            if bias_relu_col is not None:
                rb = cpool.tile([128, 1], F32, tag=f"{pfx}_rb")
                nc.scalar.activation(rb[:], bias_relu_col[:], AF.Relu)
                rb2 = cpool.tile([128, 1], F32, tag=f"{pfx}_rb2")
                nc.scalar.activation(rb2[:], rb[:], AF.Square)
                nc.vector.tensor_scalar(out=m_c[:], in0=rb[:], scalar1=float(-n_pad),
                                        scalar2=None, op0=ALU.mult)
                nc.vector.tensor_tensor(out=m_c[:], in0=m_c[:], in1=sum_c, op=ALU.add)
                nc.vector.tensor_scalar(out=m_c[:], in0=m_c[:], scalar1=1.0 / n_real,
                                        scalar2=None, op0=ALU.mult)
                nc.vector.tensor_scalar(out=q_c[:], in0=rb2[:], scalar1=float(-n_pad),
                                        scalar2=None, op0=ALU.mult)
                nc.vector.tensor_tensor(out=q_c[:], in0=q_c[:], in1=sq_c, op=ALU.add)
                nc.vector.tensor_scalar(out=q_c[:], in0=q_c[:], scalar1=1.0 / n_real,
                                        scalar2=None, op0=ALU.mult)
            else:
                nc.vector.tensor_scalar(out=m_c[:], in0=sum_c, scalar1=1.0 / n_real,
                                        scalar2=None, op0=ALU.mult)
                nc.vector.tensor_scalar(out=q_c[:], in0=sq_c, scalar1=1.0 / n_real,
                                        scalar2=None, op0=ALU.mult)
            nc.scalar.activation(t1[:], m_c[:], AF.Square)
            nc.vector.tensor_tensor(out=t1[:], in0=q_c[:], in1=t1[:], op=ALU.subtract)
            nc.vector.tensor_scalar(out=t1[:], in0=t1[:], scalar1=float(EPS),
                                    scalar2=None, op0=ALU.add)
            nc.vector.reciprocal(t1[:], t1[:])
            nc.scalar.activation(t1[:], t1[:], AF.Sqrt)
            nc.vector.tensor_tensor(out=a_c[:], in0=gc[:], in1=t1[:], op=ALU.mult)
            nc.vector.tensor_tensor(out=c_c[:], in0=m_c[:], in1=a_c[:], op=ALU.mult)
            nc.vector.tensor_tensor(out=c_c[:], in0=bec[:], in1=c_c[:], op=ALU.subtract)
            return a_c, c_c

        a1c, c1c = bn_affine(st1_sb[:, 0:1], st1_sb[:, 1:2], g1c, be1c, b1c,
                             NREAL, NPAD, "bn1")
        acc_, ccc_ = bn_affine(st1_sb[:, 2:3], st1_sb[:, 3:4], gc1c, bec1c, None,
                               N_GRAPHS, 0, "bnc")

        # W2' = a1 * W2 (fp16); r2 rows via fp32 matmul then cast
        nc.vector.tensor_scalar(out=W2f[:], in0=W2[:], scalar1=a1c[:],
                                scalar2=None, op0=ALU.mult)
        rr16 = []
        for h in range(HH):
            pr = psTRF.tile([128, 128], F32, tag="trf", space="PSUM")
            nc.tensor.matmul(pr[:1, :], lhsT=c1c[:], rhs=W2[:, h * 128:(h + 1) * 128],
                             start=True, stop=True)
            rr = cpool.tile([1, 128], F16, tag=f"r2_{h}")
            nc.vector.tensor_copy(rr[:], pr[:1, :])
            rr16.append(rr)

        # cell finish
        nc.vector.tensor_scalar(out=Wc2f[:], in0=Wc2[:], scalar1=acc_[:],
                                scalar2=None, op0=ALU.mult)
        bc2f = []
        for h in range(HH):
            pb = psTRF.tile([128, 128], F32, tag="trf", space="PSUM")
            nc.tensor.matmul(pb[:, :1], lhsT=Wc2[:, h * 128:(h + 1) * 128], rhs=ccc_[:],
                             start=True, stop=True)
            bb_ = cpool.tile([128, 1], F32, tag=f"bc2f_{h}")
            nc.vector.tensor_tensor(out=bb_[:], in0=pb[:, :1], in1=bc2c[h][:], op=ALU.add)
            bc2f.append(bb_)
        for bh in range(n_bh):
            for h in range(HH):
                px = psMM.tile([128, 512], F32, tag="mm", space="PSUM")
                nc.tensor.matmul(px[:], lhsT=Wc2f[:, h * 128:(h + 1) * 128],
                                 rhs=cT[bh][:], start=True, stop=True)
                xc = cell_p.tile([128, 512], F32, tag="cellxc")
                nc.scalar.activation(xc[:], px[:], AF.Relu, bias=bc2f[h][:])
                rtc = tr_p.tile([128, 4, 128], F32, tag="cellrt")
                for bb in range(4):
                    ptf = psTRF.tile([128, 128], F32, tag="trf", space="PSUM")
                    nc.tensor.transpose(ptf[:], xc[:, bb * 128:(bb + 1) * 128], identf[:])
                    nc.vector.tensor_copy(rtc[:, bb, :], ptf[:])
                nc.scalar.dma_start(
                    out=out[G_OUT + bh * 512:G_OUT + (bh + 1) * 512,
                            h * 128:(h + 1) * 128].rearrange("(b p) f -> p b f", p=CH),
                    in_=rtc[:])

        if parts == "l1c":
            raise _PartDone()

        # ============ LAYER 2 ============
        def l2_post(t, zT):
            zs = work_p.tile([128, TD], F16, tag="zs2")
            nc.scalar.copy(zs[:], zT)
            s_t = sf_p.tile([1, TD], F16, tag="s_t")
            nc.sync.dma_start(out=s_t[:], in_=s_flat[0:1, t * TD:(t + 1) * TD])
            rt4 = tr_p.tile([128, NSELF, HH, CH], F16, tag="l2rt")
            for h in range(HH):
                uTf = psMM.tile([128, 512], F32, tag="mm", space="PSUM")
                uT = uTf[:, :TD]
                nc.tensor.matmul(uT, lhsT=W2f[:, h * 128:(h + 1) * 128],
                                 rhs=zs[:], start=True, stop=False)
                nc.tensor.matmul(uT, lhsT=rr16[h][:],
                                 rhs=s_t[:], start=False, stop=True)
                u2t = work_p.tile([128, TD], F16, tag="u2t")
                nc.scalar.activation(u2t[:], uT, AF.Relu, bias=b2c[h][:],
                                     accum_out=st2_sum[:, t * HH + h:t * HH + h + 1])
                sq = work_p.tile([128, TD], F16, tag="sq2")
                nc.scalar.activation(sq[:], u2t[:], AF.Square,
                                     accum_out=st2_sq[:, t * HH + h:t * HH + h + 1])
                for q in range(TD // CH):
                    pt = psTR.tile([128, CH], F16, tag="tr16", space="PSUM")
                    nc.tensor.transpose(pt[:], u2t[:, q * CH:(q + 1) * CH], ident16[:])
                    nc.vector.tensor_copy(rt4[:, q, h, :], pt[:])
            nc.scalar.dma_start(
                out=u2pool[t * TD:(t + 1) * TD, :].rearrange(
                    "(q p) (h f) -> p q h f", p=CH, h=HH),
                in_=rt4[:])

        agg_layer(idx2_sb, u1_full, u1_shard, l2_post, "b")

        if parts == "nol2stats":
            raise _PartDone()
        st2_red = stats_p.tile([128, 4], F32)
        for h in range(HH):
            nc.vector.tensor_reduce(
                st2_red[:, 2 * h:2 * h + 1],
                st2_sum[:].rearrange("p (t h) -> p t h", h=HH)[:, :, h],
                mybir.AxisListType.X, ALU.add)
            nc.vector.tensor_reduce(
                st2_red[:, 2 * h + 1:2 * h + 2],
                st2_sq[:].rearrange("p (t h) -> p t h", h=HH)[:, :, h],
                mybir.AxisListType.X, ALU.add)
        nc.sync.dma_start(out=st2_in[:, :], in_=st2_red[:])
        tc.strict_bb_all_engine_barrier()
        nc.gpsimd.collective_compute(
            "AllReduce", ALU.add, replica_groups=rg,
            ins=[st2_in[:]], outs=[st2_out[:]])
        st2_sb = stats_p.tile([128, 4], F32)
        nc.sync.dma_start(out=st2_sb[:], in_=st2_out[:, :])

        a2c, c2c = [], []
        for h in range(HH):
            a_, c_ = bn_affine(st2_sb[:, 2 * h:2 * h + 1], st2_sb[:, 2 * h + 1:2 * h + 2],
                               g2c[h], be2c[h], b2c[h], NREAL, NPAD, f"bn2_{h}")
            a2c.append(a_)
            c2c.append(c_)

        if parts == "nopool":
            raise _PartDone()
        # ============ POOL ============
        pooled = stats_p.tile([128, HH, G_PAD], F32)
        for (n_idx, n_g, L, col_off, out_off) in cfg["pool_ops"]:
            pidx = pool_p.tile([128, n_idx // 16], I16, tag="pidx")
            nc.sync.dma_start(out=pidx[:], in_=pool_idx[:, col_off:col_off + n_idx // 16])
            gath = pool_p.tile([128, HH, n_idx], F16, tag="pgath")
            nc.gpsimd.dma_gather(
                gath[:], u2pool[:, :], pidx[:], n_idx, n_idx,
                elem_size=OUT, transpose=True, single_packet=False)
            nc.vector.tensor_reduce(
                pooled[:, :, out_off:out_off + n_g],
                gath[:].rearrange("p h (g l) -> p h g l", g=n_g),
                mybir.AxisListType.X, ALU.max)
        for h in range(HH):
            nc.vector.tensor_scalar(
                out=pooled[:, h, :], in0=pooled[:, h, :],
                scalar1=a2c[h][:], scalar2=c2c[h][:], op0=ALU.mult, op1=ALU.add)
        for h in range(HH):
            for q in range(G_OUT // 128):
                n_here = min(128, G_PAD - q * 128)
                if n_here <= 0:
                    break
                ptf = psTRF.tile([128, 128], F32, tag="trf", space="PSUM")
                nc.tensor.transpose(ptf[:n_here, :], pooled[:, h, q * 128:q * 128 + n_here],
                                    identf[:])
                rt = tr_p.tile([128, 128], F32, tag="poolrt")
                nc.vector.tensor_copy(rt[:n_here, :], ptf[:n_here, :])
                nc.sync.dma_start(
                    out=out[q * 128:q * 128 + n_here, h * 128:(h + 1) * 128],
                    in_=rt[:n_here, :])

      except _PartDone:
          pass
    nc.compile()
    return nc


_BUILD_CACHE = {}


def _run(inputs, trace=False):
    plan = build_plan(inputs["drug_adj"], inputs["ibatch"])
    cfg, in_maps, assemble = make_cfg_inputs(plan, inputs)
    key = (cfg["NT"], cfg["C"], cfg["G_PAD"], tuple(map(tuple, cfg["pool_ops"])))
    if key not in _BUILD_CACHE:
        _BUILD_CACHE[key] = build_kernel(cfg)
    nc = _BUILD_CACHE[key]
    res = run_bass_kernel_spmd(nc, in_maps, core_ids=list(range(8)), trace=trace)
    x_drug, x_cell = assemble(res.results)
    return (x_drug, x_cell), res


def kernel(**inputs):
    inputs = {k: np.asarray(v) for k, v in inputs.items()}
    (x_drug, x_cell), _ = _run(inputs, trace=False)
    return x_drug, x_cell
